# revision 65
# baseline (speedup 1.0000x reference)
"""Trainium2 Bass kernel for nn_Chambers (6-tower MLP + coupled sigmoid recurrence).

Data-parallel over 8 NeuronCores: each core processes a 16384-sample shard in
16 chunks of 1024 samples. res tiles are PE-transposed (fp32, exact) into
[100, 512] activation halves; the 4 MLP layers run as fp32r matmuls (full PE
rate at N=512) with chambers packed block-diagonally in L3; L4 accumulates
into a persistent [96, 1024] PSUM tile using per-chunk W4 stacks whose output
column block is 6*chunk (rows outside the block accumulate zeros), which
sidesteps the engines' partition-offset alignment restriction. ACT applies
SiLU+bias straight out of PSUM. The 5-step coupled sigmoid recurrence runs on
the resident [96, 1024] raw tile via a block-diagonal [96,96] matmul.

Sync discipline: this walrus build allows at most 1 sem wait + 1 update per
engine instruction. Constants arrive in single DMAs (one fp32r pack for PE,
one fp32 pack for identity/biases); "touch" ops pre-observe DMA-lane sems;
single-dep nop chains absorb all other cross-engine and same-engine-WAW
waits so no instruction ever needs two.
"""
import numpy as np

import concourse.bass as bass
import concourse.mybir as mybir
from concourse.bass_utils import run_bass_kernel_spmd
from concourse.tile import TileContext
from concourse.tile_scheduler import N_PROCS
from concourse.vector_clock import ScopedClock
from bass_rust import add_dep_helper

F32 = mybir.dt.float32
F32R = mybir.dt.float32r
AF = mybir.ActivationFunctionType
ALU = mybir.AluOpType

# All gpsimd (SWDGE) DMAs share one completion-sem lane so consumers of the
# DMA-assembled raw tile carry a single wait.
import concourse.tile_sem_assignment as _tsa
if not getattr(_tsa.TileClockTick, "_single_swdge", False):
    _orig_tick_init = _tsa.TileClockTick.__init__

    def _tick_init(self, *a, **kw):
        _orig_tick_init(self, *a, **kw)
        self.swdge_sem_count = 1

    _tsa.TileClockTick.__init__ = _tick_init
    _tsa.TileClockTick._single_swdge = True

B = 131072
NCORES = 8
BS = B // NCORES           # 16384 samples per core
T = 1024                   # chunk (samples)
NCH = BS // T              # 16 chunks
RES_DIM = 100
CF_ITERS = 5
CF_K = 0.02

# wf (fp32) column layout
IDC = 0        # identity [128,128]
B1C = 128      # 6 cols
B2C = 134      # 6 cols ([0:64] per chamber)
B3C = 140      # 3 cols ([0:64] per pair)
B4C = 143      # 1 col (b4 tiled x16 over 96 rows)
B2PC = 144     # 3 cols (pair-packed b2: rows 0:64=b2[2pr], 64:128=b2[2pr+1])
B3PC = 147     # 1 col (b3 chambers 0-3 by 32s)
FCOLS = 148

# wr (fp32r) column layout
W1C = 0                    # 6*128
W2C = 768                  # 6*64
W3C = 1152                 # 3*64
CDC = 1344                 # 96
I96C = 1440                # 96 (identity, for raw+delta accumulate)
W4AC = 1536                # 16*96 (per-chunk stacks, chambers 0-3)
W4BC = W4AC + 16 * 96      # 16*96 (per-chunk stacks, chambers 4-5)
W2BC = W4BC + 16 * 96      # 3*128: odd-chamber W2 shifted to out rows 64:127
W3BC = W2BC + 3 * 128      # 128: pair-1 W3 shifted to out rows 64:127
RCOLS = W3BC + 128


class TC(TileContext):
    """TileContext with a walrus-compatible epilogue (split final waits)."""

    def _drain_and_barrier(self, tick_clock, wait_clock):
        nc = self.nc
        full = ScopedClock({None: tick_clock.global_clock})
        for scope, vc in full.items():
            for proc in range(N_PROCS):
                t = vc.peek_next(proc) - 1
                if t > 0:
                    sc = ScopedClock()
                    sc.require_at_least(scope, proc, t)
                    w = nc.sync.nop(nofuse=True)
                    wait_clock.add_sem_waits(w.ins, sc)
        for eng in nc.engines.values():
            eng.drain(fusable=False)
        nc.all_engine_barrier(sem_only=True)
        assert self.sems is not None
        popped = nc._tile_sem_poison_stack.pop()
        assert popped is self._sem_poison
        nc.clear_and_free_semaphores(list(self.sems.allocated().values()))
        for eng in nc.engines.values():
            eng.drain(fusable=False)
        nc.all_engine_barrier(sem_only=True)


def _absorb(eng, deps, after=None):
    """Chain of single-wait nops on `eng`, ordered after `after` if given.
    Returns the last nop (or `after` if no deps)."""
    last = after
    for d in deps:
        if d is None:
            continue
        n = eng.nop(nofuse=True)
        add_dep_helper(n.ins, d.ins, sync=True, reason="absorb")
        if last is not None:
            add_dep_helper(n.ins, last.ins, sync=False, reason="absorb-chain")
        last = n
    return last


def _order(after_inst, before_inst):
    if after_inst is not None and before_inst is not None:
        add_dep_helper(after_inst.ins, before_inst.ins, sync=False, reason="order")


def build_module():
    nc = bass.Bass()
    res_d = nc.dram_tensor("res", [BS, RES_DIM], F32, kind="ExternalInput")
    wf_d = nc.dram_tensor("wf", [128, FCOLS], F32, kind="ExternalInput")
    wr_d = nc.dram_tensor("wr", [128, RCOLS], F32R, kind="ExternalInput")
    raw_d = nc.dram_tensor("raw_out", [96, T], F32, kind="ExternalOutput")
    act_d = nc.dram_tensor("act_out", [96, T], F32, kind="ExternalOutput")

    MMB = 3  # bufs on the shared matmul psum tag

    with TC(nc) as tc:
        with (
            tc.tile_pool(name="wconst", bufs=1) as wpool,
            tc.tile_pool(name="sbres", bufs=1) as sbres,
            tc.tile_pool(name="sbrt", bufs=4) as sbrt,
            tc.tile_pool(name="sbh", bufs=2) as sbh,
            tc.tile_pool(name="sbrec", bufs=1) as sbrec,
            tc.tile_pool(name="pstr", bufs=1, space="PSUM") as pstr,
            tc.tile_pool(name="psscr", bufs=1, space="PSUM") as psscr,
            tc.tile_pool(name="psmm", bufs=MMB, space="PSUM") as psmm,
        ):
            # DMA issue order matters: chunk-0 res and the L1 weights
            # first so compute starts ~4us in; the bulky remainder of the
            # weight pack and later res chunks stream behind.
            res_sb0 = wpool.tile([128, 8 * RES_DIM], F32)
            nc.sync.dma_start(
                out=res_sb0[:],
                in_=res_d[0:T].rearrange("(p n) d -> p (n d)", p=128))
            wf = wpool.tile([128, FCOLS], F32)
            nc.sync.dma_start(out=wf[:], in_=wf_d[:])
            wr = wpool.tile([128, RCOLS], F32R)
            nc.sync.dma_start(out=wr[:, 0:W2C], in_=wr_d[:, 0:W2C])
            res_sb1 = wpool.tile([128, 3 * 8 * RES_DIM], F32)
            nc.sync.dma_start(
                out=res_sb1[:],
                in_=res_d[T:4 * T].rearrange("(p n) d -> p (n d)", p=128))
            nc.sync.dma_start(out=wr[:, W2C:], in_=wr_d[:, W2C:])
            res_sb = wpool.tile([128, (NCH - 4) * 8 * RES_DIM], F32)
            nc.sync.dma_start(
                out=res_sb[:],
                in_=res_d[4 * T:].rearrange("(p n) d -> p (n d)", p=128))
            ident = wf[:, IDC:IDC + 128]

            raw_sb = sbrec.tile([96, T], F32)
            act_r = sbrec.tile([96, T], F32R)
            tmp_sb = sbrec.tile([96, T], F32)
            act_o = sbrec.tile([96, T], F32)
            scr = sbrec.tile([1, 2], F32)
            scrA = sbrec.tile([1, 512], F32)
            scrA2 = sbrec.tile([96, 16], F32)
            scrP = sbrec.tile([1, 16], F32)
            scrD = sbrec.tile([1, 128], F32)

            ps_scr = psscr.tile([128, 512], F32)  # row 0: touch scratch cells

            # PE touch ops: observe the two const DMA lanes (1 wait each)
            warm_r = nc.tensor.matmul(ps_scr[0:1, 496:498], wr[0:1, 0:1],
                                      wr[0:1, 0:2], start=True, stop=True)
            warm_f = nc.tensor.matmul(ps_scr[0:1, 498:500], wf[0:1, 0:1],
                                      wf[0:1, 0:2], start=True, stop=True)
            _order(warm_f, warm_r)
            # ACT touch op: observe the wf DMA lane
            nc.scalar.activation(scr[0:1, 0:1], wf[0:1, B1C:B1C + 1], AF.Copy)

            # Rolling state. Rule: each instruction carries at most one
            # sem wait (its own-engine wait); every cross-engine dependency
            # is pre-observed by a real "touch" instruction (1x2 matmul on
            # PE, 1-elem copy/activation on DVE/ACT) reading the producer's
            # tile. PSUM matmul tiles are [128,1024] (2 banks) on two
            # rotating single-buffer tags: slot reuse is deterministic
            # (k-2) and the pre-touch waits on a silu that has already
            # retired, so ACT streams back-to-back.
            tr_state = []
            pe_tail = warm_f
            act_tail = None
            dve_tail = None
            tcol = [0]
            acol = [0]
            dcol = [0]

            def pe_touch(src_ap):
                nonlocal pe_tail
                t = tcol[0]; tcol[0] += 1
                assert t < 248
                col = 2 * t
                m = nc.tensor.matmul(ps_scr[0:1, col:col + 2],
                                     src_ap[:, 0:1], src_ap[:, 0:2],
                                     start=True, stop=True)
                _order(m, pe_tail)
                pe_tail = m
                return m

            def act_touch(src_ap):
                nonlocal act_tail
                t = acol[0]; acol[0] += 1
                s = nc.scalar.activation(scrA[0:1, t:t + 1], src_ap, AF.Copy)
                _order(s, act_tail)
                act_tail = s
                return s

            def dve_touch(src_ap):
                nonlocal dve_tail
                t = dcol[0]; dcol[0] += 1
                c = nc.vector.tensor_copy(scrD[0:1, t:t + 1], src_ap)
                _order(c, dve_tail)
                dve_tail = c
                return c

            tag_rr = [0]
            tag_state = [None, None, None]

            def new_mm_tile(name, touch=True, width=T):
                nonlocal pe_tail
                tg = tag_rr[0] % 3
                tag_rr[0] += 1
                st = tag_state[tg]
                if st is not None:
                    if touch:
                        tile_, row_, col_ = st
                        pe_touch(tile_[row_:row_ + 1, col_:col_ + 2])
                    tag_state[tg] = None
                t = psmm.tile([128, width], F32, tag=f"mm{tg}", bufs=1,
                              name=name)
                return t, tg

            def mm(out_ap, lhs_ap, rhs_ap, **kw):
                nonlocal pe_tail
                m = nc.tensor.matmul(out_ap, lhs_ap, rhs_ap, **kw)
                _order(m, pe_tail)
                pe_tail = m
                return m

            def set_act_tail(s):
                nonlocal act_tail
                act_tail = s

            def silu(out_ap, pm_ap, bias_ap, out_tile, tg, row, col):
                nonlocal act_tail, act_tile
                s = nc.scalar.activation(out_ap, pm_ap, AF.Silu, bias=bias_ap)
                _order(s, act_tail)
                act_tail = s
                act_tile = out_tile
                tag_state[tg] = (out_tile, row, col)
                return s

            act_tile = None

            def emit_tr(i):
                nonlocal pe_tail, dve_tail
                if i == 0:
                    rq, coff = res_sb0, 0
                elif i < 4:
                    rq, coff = res_sb1, (i - 1) * 8 * RES_DIM
                else:
                    rq, coff = res_sb, (i - 4) * 8 * RES_DIM
                if i in (1, 4):
                    cell = 504 if i == 1 else 508
                    m_ = nc.tensor.matmul(ps_scr[0:1, cell:cell + 2],
                                          rq[0:1, 0:1], rq[0:1, 0:2],
                                          start=True, stop=True)
                    _order(m_, pe_tail)
                    pe_tail = m_
                rTs = []
                for h in range(2):
                    ptr = pstr.tile([100, 512], F32, tag="tr", name="ptr")
                    last_t = None
                    for n in range(4):
                        nn_ = 4 * h + n
                        t_ = nc.tensor.transpose(
                            ptr[:, n * 128:(n + 1) * 128],
                            rq[:, coff + nn_ * RES_DIM:coff + (nn_ + 1) * RES_DIM],
                            ident,
                        )
                        _order(t_, pe_tail)
                        pe_tail = t_
                        last_t = t_
                    rT = sbrt.tile([100, 512], F32R, tag="rT", name="rT")
                    dve_touch(ptr[0:1, 0:1])
                    cp = nc.vector.tensor_copy(rT[:], ptr[:])
                    _order(cp, dve_tail)
                    dve_tail = cp
                    tr_state.append((last_t, cp))
                    rTs.append(rT)
                    pe_touch(rT[0:1, 0:2])
                return rTs

            rts_next = emit_tr(0)
            pending_l4 = []
            for i in range(NCH):
                rTs = rts_next

                # L1: 3 chamber-pairs, one [128,1024] tile per chamber
                h1s = []
                for cp in range(3):
                    ha = sbh.tile([128, T], F32R, tag="h1", bufs=7, name="h1a")
                    hb = sbh.tile([128, T], F32R, tag="h1", bufs=7, name="h1b")
                    pa, ta = new_mm_tile("pm1", touch=False)
                    pb, tb = new_mm_tile("pm1")
                    for s in range(2):
                        mm(pa[:, s * 512:(s + 1) * 512],
                           wr[0:100, W1C + 2 * cp * 128:W1C + (2 * cp + 1) * 128],
                           rTs[s][:], start=True, stop=True)
                    for s in range(2):
                        mm(pb[:, s * 512:(s + 1) * 512],
                           wr[0:100, W1C + (2 * cp + 1) * 128:W1C + (2 * cp + 2) * 128],
                           rTs[s][:], start=True, stop=True)
                    act_touch(pb[0:1, 512:513])
                    silu(ha[:], pa[:], wf[:, B1C + 2 * cp:B1C + 2 * cp + 1],
                         ha, ta, 0, 0)
                    silu(hb[:], pb[:], wf[:, B1C + 2 * cp + 1:B1C + 2 * cp + 2],
                         hb, tb, 0, 0)
                    h1s.extend([ha, hb])



                # L2: per pair, one [64,1024] region per chamber
                if i == 0:
                    # observe the second wr segment's lane just before L2
                    # first needs it (keeps it off the startup critical path)
                    w2 = nc.tensor.matmul(ps_scr[0:1, 492:494],
                                          wr[0:1, W2C:W2C + 1],
                                          wr[0:1, W2C:W2C + 2],
                                          start=True, stop=True)
                    _order(w2, pe_tail)
                    pe_tail = w2
                h2s = []
                l2t = []
                for pr in range(3):
                    pm2, tg2 = new_mm_tile("pm2")
                    for s in range(2):
                        mm(pm2[:, s * 512:(s + 1) * 512],
                           wr[:, W2BC + pr * 128:W2BC + (pr + 1) * 128],
                           h1s[2 * pr + 1][:, s * 512:(s + 1) * 512],
                           start=True, stop=False)
                        mm(pm2[0:64, s * 512:(s + 1) * 512],
                           wr[:, W2C + 2 * pr * 64:W2C + (2 * pr + 1) * 64],
                           h1s[2 * pr][:, s * 512:(s + 1) * 512],
                           start=False, stop=True)
                    l2t.append((pm2, tg2))
                for pr in range(3):
                    pm2, tg2 = l2t[pr]
                    if pr == 0:
                        act_touch(pm2[0:1, 512:513])
                    h2 = sbh.tile([128, T], F32R, tag="h2", bufs=4, name="h2")
                    silu(h2[:], pm2[:], wf[:, B2PC + pr:B2PC + pr + 1],
                         h2, tg2, 0, 0)
                    h2s.append(h2)

                if i + 1 < NCH:
                    rts_next = emit_tr(i + 1)
                if pending_l4:
                    pending_l4.pop(0)()
                # L3: pairs 0,1 merged into one tile; pair 2 separate
                h3a = sbh.tile([128, T], F32R, tag="h3", bufs=4, name="h3a")
                h3b = sbh.tile([128, T], F32R, tag="h3", bufs=4, name="h3b")
                pa, ta = new_mm_tile("pm3", touch=False)
                pc, tc_ = new_mm_tile("pm3b")
                for s in range(2):
                    mm(pa[:, s * 512:(s + 1) * 512],
                       wr[:, W3BC:W3BC + 128],
                       h2s[1][:, s * 512:(s + 1) * 512], start=True, stop=False)
                    mm(pa[0:64, s * 512:(s + 1) * 512],
                       wr[:, W3C:W3C + 64],
                       h2s[0][:, s * 512:(s + 1) * 512], start=False, stop=True)
                pe_touch(h2s[2][0:1, 0:2])  # newest h2 silu
                for s in range(2):
                    mm(pc[0:64, s * 512:(s + 1) * 512],
                       wr[:, W3C + 128:W3C + 192],
                       h2s[2][:, s * 512:(s + 1) * 512], start=True, stop=True)
                act_touch(pc[0:1, 512:513])
                silu(h3a[:], pa[:], wf[:, B3PC:B3PC + 1], h3a, ta, 0, 0)
                silu(h3b[0:64, :], pc[0:64, :], wf[0:64, B3C + 2:B3C + 3],
                     h3b, tc_, 0, 0)

                # L4 deferred past the next chunk's L1 block: per-chunk
                # [6,T] raw rows land in a rotation tile (base 0), are
                # bias-copied to SBUF by ACT, then DMA'd (single SWDGE
                # lane) into raw_sb rows 6i..6i+5.
                def emit_l4(i=i, h3a=h3a, h3b=h3b):
                    pe_touch(h3b[0:1, 0:2])   # h3 silus retired by now
                    pm4, tg4 = new_mm_tile("pm4")
                    for s in range(2):
                        mm(pm4[0:6, s * 512:(s + 1) * 512],
                           wr[:, W4AC:W4AC + 6],
                           h3a[:, s * 512:(s + 1) * 512],
                           start=True, stop=False)
                        mm(pm4[0:6, s * 512:(s + 1) * 512],
                           wr[0:64, W4BC:W4BC + 6],
                           h3b[0:64, s * 512:(s + 1) * 512],
                           start=False, stop=True)
                    act_touch(pm4[0:1, 512:513])
                    raw_i = sbh.tile([6, T], F32, tag="rawi", bufs=2,
                                     name="raw_i")
                    ro = nc.scalar.activation(raw_i[:], pm4[0:6, :],
                                              AF.Identity,
                                              bias=wf[0:6, B4C:B4C + 1])
                    _order(ro, act_tail)
                    set_act_tail(ro)
                    tag_state[tg4] = (raw_i, 0, 0)
                    # ACT observes the assembly DMAs (covers the raw_i slot
                    # WAR two chunks later); Pool observes ACT through it
                    s_ = nc.scalar.activation(scrA2[:, (i % 16):(i % 16) + 1],
                                              raw_sb[0:96, 0:1], AF.Copy)
                    _order(s_, act_tail)
                    set_act_tail(s_)
                    nc.gpsimd.tensor_copy(scrP[0:1, (i % 16):(i % 16) + 1],
                                          scrA2[0:1, (i % 16):(i % 16) + 1])
                    nc.gpsimd.dma_start(out=raw_sb[6 * i:6 * i + 6, :],
                                        in_=raw_i[:])
                pending_l4.append(emit_l4)

            if pending_l4:
                pending_l4.pop(0)()

            # ---- coupled sigmoid recurrence on [96, T] ----
            raw_r = sbrec.tile([96, T], F32R)
            cpr = nc.vector.tensor_copy(raw_r[:], raw_sb[:])
            _order(cpr, dve_tail)
            dve_tail = cpr
            pe_touch(raw_r[0:1, 0:2])
            sig = nc.scalar.activation(act_r[:], raw_sb[:], AF.Sigmoid)
            _order(sig, act_tail)
            act_tail = sig
            for kk in range(CF_ITERS):
                dst = act_r if kk < CF_ITERS - 1 else act_o
                pe_touch(act_r[0:1, 0:2])   # PE observes the latest sigmoid
                for s in range(2):
                    pm5, tg5 = new_mm_tile("pm5", touch=False, width=512)
                    mm(pm5[0:96, 0:512],
                       wr[0:96, CDC:CDC + 96],
                       act_r[:, s * 512:(s + 1) * 512],
                       start=True, stop=False)
                    mm(pm5[0:96, 0:512],
                       wr[0:96, I96C:I96C + 96],
                       raw_r[:, s * 512:(s + 1) * 512],
                       start=False, stop=True)
                    act_touch(pm5[0:1, 0:1])
                    sig = nc.scalar.activation(
                        dst[:, s * 512:(s + 1) * 512], pm5[0:96, 0:512],
                        AF.Sigmoid)
                    _order(sig, act_tail)
                    act_tail = sig
                    tag_state[tg5] = (dst, 0, s * 512)

            nc.sync.dma_start(out=raw_d[:], in_=raw_sb[:])
            nc.sync.dma_start(out=act_d[:], in_=act_o[:])

    return nc


def _pack_consts(W1, b1, W2, b2, W3, b3, W4, b4, coupling, decay):
    wf = np.zeros((128, FCOLS), dtype=np.float32)
    wf[:, IDC:IDC + 128] = np.eye(128, dtype=np.float32)
    for c in range(6):
        wf[:, B1C + c] = b1[c]
    for c in range(6):
        wf[0:64, B2C + c] = b2[c]
    for pr in range(3):
        wf[0:32, B3C + pr] = b3[2 * pr]
        wf[32:64, B3C + pr] = b3[2 * pr + 1]
    wf[0:96, B4C] = np.tile(b4, 16)

    wr = np.zeros((128, RCOLS), dtype=np.float32)
    for c in range(6):
        wr[0:100, W1C + c * 128:W1C + (c + 1) * 128] = W1[c]
        wr[0:128, W2C + c * 64:W2C + (c + 1) * 64] = W2[c]
    for pr in range(3):
        wr[0:64, W3C + pr * 64:W3C + pr * 64 + 32] = W3[2 * pr]
        wr[64:128, W3C + pr * 64 + 32:W3C + (pr + 1) * 64] = W3[2 * pr + 1]
    cd = (decay[:, None] * coupling * CF_K).astype(np.float32)
    for g in range(16):
        wr[6 * g:6 * g + 6, CDC + 6 * g:CDC + 6 * g + 6] = cd
    wr[0:96, I96C:I96C + 96] = np.eye(96, dtype=np.float32)
    for c in range(4):
        wr[c * 32:(c + 1) * 32, W4AC + c] = W4[c]
    for c2 in range(2):
        wr[c2 * 32:(c2 + 1) * 32, W4BC + 4 + c2] = W4[4 + c2]
    # odd chambers of each L2 pair, shifted to output rows 64:127 (cols
    # 0:64 stay zero so start=True clears the even chamber's rows for the
    # accumulating second matmul)
    for pr in range(3):
        wr[0:128, W2BC + pr * 128 + 64:W2BC + (pr + 1) * 128] = W2[2 * pr + 1]
        wf[0:64, B2PC + pr] = b2[2 * pr]
        wf[64:128, B2PC + pr] = b2[2 * pr + 1]
    # L3 pair 1 (chambers 2,3) shifted to rows 64:127 of the merged tile
    wr[0:64, W3BC + 64:W3BC + 96] = W3[2]
    wr[64:128, W3BC + 96:W3BC + 128] = W3[3]
    for c in range(4):
        wf[c * 32:(c + 1) * 32, B3PC] = b3[c]
    return wf, wr


def _unshard(per_core, key):
    """[96, T] group layout -> [BS, 6] per core, concat to [B, 6].

    Chunk 0: sample p*8+n8. Chunks 1-3: 1024 + p*24 + (i-1)*8 + n8.
    Chunks 4-15: 4096 + p*96 + (i-4)*8 + n8."""
    outs = []
    for r in per_core:
        a = r[key].reshape(NCH, 6, 8, 128)             # [i, c, n8, p]
        out = np.empty((BS, 6), dtype=a.dtype)
        out[0:T] = a[0].transpose(2, 1, 0).reshape(T, 6)
        out[T:4 * T] = a[1:4].transpose(3, 0, 2, 1).reshape(3 * T, 6)
        out[4 * T:] = a[4:].transpose(3, 0, 2, 1).reshape(12 * T, 6)
        outs.append(out)
    return np.concatenate(outs, axis=0)


def kernel(res, W1, b1, W2, b2, W3, b3, W4, b4, coupling, decay):
    res = np.asarray(res, dtype=np.float32)
    args = [np.asarray(a, dtype=np.float32)
            for a in (W1, b1, W2, b2, W3, b3, W4, b4, coupling, decay)]
    wf, wr = _pack_consts(*args)

    nc = build_module()
    in_maps = [
        {"res": np.ascontiguousarray(res[i * BS:(i + 1) * BS]), "wf": wf, "wr": wr}
        for i in range(NCORES)
    ]
    results = run_bass_kernel_spmd(nc, in_maps, core_ids=list(range(NCORES)))
    act = _unshard(results.results, "act_out")
    raw = _unshard(results.results, "raw_out")
    return act, raw
